# revision 1
# baseline (speedup 1.0000x reference)
"""Trainium2 Bass kernel for nn_DotAttention (B=8 data-parallel over 8 cores).

Per core (one batch element):
  xp = relu(x @ Wi + bi)            [2048, 96]
  mp = relu(m @ Wm + bm)            [2048, 96]
  S.T[jm, jx] = mp[jm,:] . xp[jx,:]             (PE, fp16 operands)
  E = exp(S.T / sqrt(96) + maskbias[jm])        (ACT, mask folded into bias)
  U.T[d, jx] = sum_jm mtilde[jm, d] * E[jm, jx] (PE; mtilde = [m | 1] so row 150
                                                 of U.T is the softmax denom)
  out.T = sigmoid(Wg.T @ res.T + bg) * res.T,  res.T = [x.T ; U.T / denom]
  out = PE-transpose(out.T)  -> DMA

All matmul operands are fp16 (PSUM accumulation is fp32); data paths that
reach the output directly (x, U, gates multiplied by res) stay fp32.

Everything is transposed on-chip ("T layout": feature dim on partitions)
because the PE contracts over the partition dim; x/m are transposed on entry
via PE-transpose, the output is transposed back at the end.
"""

import math

import numpy as np

import concourse.bass as bass
import concourse.mybir as mybir
import concourse.tile as tile
from concourse import bacc
from concourse.bass_utils import run_bass_kernel_spmd
from concourse.masks import make_identity

F32 = mybir.dt.float32
F16 = mybir.dt.float16
I32 = mybir.dt.int32

B = 8
JX = 2048
JM = 2048
D = 150
H = 96
G = 300
NCH = JX // 128  # 16 column chunks of the transposed layout
NJT = JM // 128  # 16 key tiles
HALF = 1024  # jx processed in two halves (PSUM budget)
NSUB = HALF // 512  # matmul free-dim is limited to 512 (one PSUM bank)
SCALE = 1.0 / math.sqrt(float(H))
NEG_BIG = 1.0e30


def _body(tc, x_d, m_d, mask_d, wi_d, bi_d, wm_d, bm_d, wg_d, bg_d, o_d):
    nc = tc.nc
    Relu = mybir.ActivationFunctionType.Relu
    Exp = mybir.ActivationFunctionType.Exp
    Sigmoid = mybir.ActivationFunctionType.Sigmoid

    import contextlib

    with contextlib.ExitStack() as ctx:
        const = ctx.enter_context(tc.tile_pool(name="const", bufs=1))
        work = ctx.enter_context(tc.tile_pool(name="work", bufs=2))
        epool = ctx.enter_context(tc.tile_pool(name="epool", bufs=8))
        psb = ctx.enter_context(tc.tile_pool(name="psb", bufs=2, space="PSUM"))
        psu = ctx.enter_context(tc.tile_pool(name="psu", bufs=1, space="PSUM"))

        ident = const.tile([128, 128], F32)
        make_identity(nc, ident)
        ident16 = const.tile([128, 128], F16)
        make_identity(nc, ident16)

        def warm_mm(n):
            jp = psb.tile([128, 128], F32, tag="big", name="junk")
            for _ in range(n):
                nc.tensor.matmul(
                    jp, ident16, ident16, start=True, stop=True,
                    skip_group_check=True,
                )

        # preload the exp/relu ACT table set during the input DMA wait
        dummy = const.tile([1, 1], F32)
        nc.scalar.activation(
            out=dummy, in_=ident[0:1, 0:1],
            func=mybir.ActivationFunctionType.Exp, scale=1.0,
        )

        # ---- input loads -------------------------------------------------
        # mtilde in fp16, natural layout (jm on partitions); columns are
        # [m | 0-pad | 1]. Engine APs must start at a 32-aligned partition,
        # so the all-ones column (softmax denominator) is padded out to
        # column 160 -> U2 partition 32. Loaded first (gates m transposes
        # and the attention stationaries); split into chunks so transposes
        # can start before the whole tensor lands.
        mt16 = const.tile([128, NJT, 162], F16)
        m_nat = const.tile([128, NJT, D], F32)
        m_re = m_d.rearrange("(n p) d -> p n d", p=128)
        for q in range(8):
            qs = slice(q * NJT // 8, (q + 1) * NJT // 8)
            dq = nc.scalar if q % 2 == 0 else nc.sync
            dq.dma_start(out=m_nat[:, qs, :], in_=m_re[:, qs, :])
            nc.vector.tensor_copy(out=mt16[:, qs, 0:D], in_=m_nat[:, qs, :])
        nc.vector.memset(mt16[:, :, D:160], 0.0)
        nc.vector.memset(mt16[:, :, 160:161], 1.0)
        x_nat = const.tile([128, NCH, D], F32)
        x_re = x_d.rearrange("(n p) d -> p n d", p=128)
        for q in range(8):
            qs = slice(q * NCH // 8, (q + 1) * NCH // 8)
            dq = nc.sync if q % 2 == 0 else nc.scalar
            dq.dma_start(out=x_nat[:, qs, :], in_=x_re[:, qs, :])

        # mask -> per-partition additive bias for exp: (mask-1)*1e30
        mask_sb = const.tile([NJT, 128], I32)
        nc.sync.dma_start(out=mask_sb, in_=mask_d.rearrange("(n p) -> n p", p=128))
        maskf = const.tile([NJT, 128], F32)
        nc.vector.tensor_copy(out=maskf, in_=mask_sb)
        nc.vector.tensor_scalar(
            out=maskf,
            in0=maskf,
            scalar1=1.0,
            scalar2=NEG_BIG,
            op0=mybir.AluOpType.subtract,
            op1=mybir.AluOpType.mult,
        )
        mb_ps = psb.tile([128, NJT], F32, tag="big")
        nc.tensor.transpose(mb_ps, maskf, ident[:NJT, :NJT])
        maskbias = const.tile([128, NJT], F32)
        nc.vector.tensor_copy(out=maskbias, in_=mb_ps)

        # ---- weights -----------------------------------------------------
        wstage = const.tile([128, 2 * H], F32)
        nc.sync.dma_start(out=wstage[:, 0:H], in_=wi_d[0:128, :])
        nc.sync.dma_start(out=wstage[:, H : 2 * H], in_=wm_d[0:128, :])
        wstage2 = const.tile([D - 128, 2 * H], F32)
        nc.sync.dma_start(out=wstage2[:, 0:H], in_=wi_d[128:D, :])
        nc.sync.dma_start(out=wstage2[:, H : 2 * H], in_=wm_d[128:D, :])
        wi16a = const.tile([128, H], F16)
        nc.vector.tensor_copy(out=wi16a, in_=wstage[:, 0:H])
        wi16b = const.tile([D - 128, H], F16)
        nc.vector.tensor_copy(out=wi16b, in_=wstage2[:, 0:H])
        wm16a = const.tile([128, H], F16)
        nc.vector.tensor_copy(out=wm16a, in_=wstage[:, H : 2 * H])
        wm16b = const.tile([D - 128, H], F16)
        nc.vector.tensor_copy(out=wm16b, in_=wstage2[:, H : 2 * H])
        bi_sb = const.tile([H, 1], F32)
        nc.sync.dma_start(out=bi_sb, in_=bi_d.rearrange("(n one) -> n one", one=1))
        bm_sb = const.tile([H, 1], F32)
        nc.sync.dma_start(out=bm_sb, in_=bm_d.rearrange("(n one) -> n one", one=1))
        wg16 = []
        for gi, (g0, g1) in enumerate([(0, 128), (128, D), (D, D + 128), (D + 128, G)]):
            wst = const.tile(
                [g1 - g0, G], F32, tag=f"wgst_{gi}", name=f"wgst_{gi}"
            )
            nc.sync.dma_start(out=wst, in_=wg_d[g0:g1, :])
            w = const.tile([g1 - g0, G], F16, tag=f"wg16_{gi}", name=f"wg16_{gi}")
            nc.vector.tensor_copy(out=w, in_=wst)
            wg16.append(w)
        bg_sb = []
        for gi, (g0, g1) in enumerate([(0, 128), (128, 256), (256, G)]):
            t = const.tile([g1 - g0, 1], F32, tag=f"bg_{gi}", name=f"bg_{gi}")
            nc.sync.dma_start(
                out=t, in_=bg_d[g0:g1].rearrange("(n one) -> n one", one=1)
            )
            bg_sb.append(t)
        ones16 = const.tile([1, 128], F16)
        nc.vector.memset(ones16, 1.0)

        # ---- transpose x and m into T layout -----------------------------
        xT1 = const.tile([128, JX], F32)
        xT2 = const.tile([D - 128, JX], F32)
        xT116 = const.tile([128, JX], F16)
        xT216 = const.tile([D - 128, JX], F16)
        mT116 = const.tile([128, JM], F16)
        mT216 = const.tile([D - 128, JM], F16)
        def tp_tile(i, shape, dt, name):
            kind = ["big", "big", "u1", "u2"][i % 4]
            pool = psb if kind == "big" else psu
            return pool.tile(shape, dt, tag=kind, name=name)

        # batch 4 chunk-transposes into one PSUM tile + one copy: cuts the
        # copy/semaphore count 4x (the in-transpose phase is latency-bound
        # on the copy chain, not on the PE)
        ti = 0
        for c4 in range(0, NJT, 4):
            # m group
            sl4 = slice(c4 * 128, (c4 + 4) * 128)
            eng = nc.vector if (c4 // 4) % 2 == 0 else nc.scalar
            t1 = tp_tile(ti, [128, 512], F16, "mtp1"); ti += 1
            t2 = tp_tile(ti, [D - 128, 512], F16, "mtp2"); ti += 1
            for i in range(4):
                ps = slice(i * 128, (i + 1) * 128)
                nc.tensor.transpose(t1[:, ps], mt16[:, c4 + i, 0:128], ident16)
                nc.tensor.transpose(t2[:, ps], mt16[:, c4 + i, 128:D], ident16)
            (eng.tensor_copy if eng is nc.vector else eng.copy)(
                out=mT116[:, sl4], in_=t1
            )
            (eng.tensor_copy if eng is nc.vector else eng.copy)(
                out=mT216[:, sl4], in_=t2
            )
            # x group
            eng = nc.scalar if (c4 // 4) % 2 == 0 else nc.vector
            t1 = tp_tile(ti, [128, 512], F32, "xtp1"); ti += 1
            t2 = tp_tile(ti, [D - 128, 512], F32, "xtp2"); ti += 1
            for i in range(4):
                ps = slice(i * 128, (i + 1) * 128)
                nc.tensor.transpose(t1[:, ps], x_nat[:, c4 + i, 0:128], ident)
                nc.tensor.transpose(t2[:, ps], x_nat[:, c4 + i, 128:D], ident)
            (eng.tensor_copy if eng is nc.vector else eng.copy)(
                out=xT1[:, sl4], in_=t1
            )
            (eng.tensor_copy if eng is nc.vector else eng.copy)(
                out=xT2[:, sl4], in_=t2
            )
            if c4 == 4:
                # h0 fp16 cast as soon as the first half's transposes land
                nc.vector.tensor_copy(out=xT116[:, 0:HALF], in_=xT1[:, 0:HALF])
                nc.vector.tensor_copy(out=xT216[:, 0:HALF], in_=xT2[:, 0:HALF])
        # ---- projections: xpT = relu(Wi.T @ x.T + bi), same for m --------
        # h-major with per-half fp16 x casts: proj(h0) only waits the first
        # half's cast, which completes while the PE finishes the transposes
        xpT16 = const.tile([H, JX], F16)
        mpT16 = const.tile([H, JM], F16)
        for h in range(2):
            hs = slice(h * HALF, (h + 1) * HALF)
            if h == 1:
                nc.vector.tensor_copy(out=xT116[:, hs], in_=xT1[:, hs])
                nc.vector.tensor_copy(out=xT216[:, hs], in_=xT2[:, hs])
            for wa, wb, bsb, srcA, srcB, dst in [
                (wi16a, wi16b, bi_sb, xT116, xT216, xpT16),
                (wm16a, wm16b, bm_sb, mT116, mT216, mpT16),
            ]:
                pp = psb.tile([H, HALF], F32, tag="big")
                for sx in range(NSUB):
                    ss = slice(h * HALF + sx * 512, h * HALF + (sx + 1) * 512)
                    ps = slice(sx * 512, (sx + 1) * 512)
                    nc.tensor.matmul(
                        pp[:, ps], wa, srcA[:, ss],
                        start=True, stop=False, skip_group_check=True,
                    )
                    nc.tensor.matmul(
                        pp[:, ps], wb, srcB[:, ss],
                        start=False, stop=True, skip_group_check=True,
                    )
                nc.scalar.activation(
                    out=dst[:, hs], in_=pp, func=Relu, bias=bsb, scale=1.0
                )

        # ---- attention: scores -> exp -> weighted sum, per jx half -------
        # then normalize, gate, transpose back and store, still per half so
        # half-0 tail work overlaps half-1 attention on other engines.
        o_re = o_d.rearrange("(n p) k -> n p k", p=128)
        kranges = [(0, 128), (128, 256), (256, G)]
        U1n, U2n, rr16n = [], [], []
        for h in range(2):
            hs = slice(h * HALF, (h + 1) * HALF)
            U1 = psu.tile([128, HALF], F32, tag="u1")
            U2 = psu.tile([33, HALF], F32, tag="u2")
            # software-pipelined: scores(j+1) is emitted before the
            # weighted-sum matmuls of j, so the PE runs a full tile ahead of
            # the ACT exp instead of stalling on it every few tiles
            def emit_scores(j):
                sp = psb.tile([128, HALF], F32, tag="big", name="sp")
                for sx in range(NSUB):
                    ss = slice(h * HALF + sx * 512, h * HALF + (sx + 1) * 512)
                    nc.tensor.matmul(
                        sp[:, sx * 512 : (sx + 1) * 512],
                        mpT16[:, j * 128 : (j + 1) * 128],
                        xpT16[:, ss],
                        start=True, stop=True, skip_group_check=True,
                    )
                return sp

            sp_cur = emit_scores(0)
            for j in range(NJT):
                sp_next = emit_scores(j + 1) if j + 1 < NJT else None
                e16 = epool.tile([128, HALF], F16, tag="e16")
                nc.scalar.activation(
                    out=e16, in_=sp_cur, func=Exp,
                    bias=maskbias[:, j : j + 1], scale=SCALE,
                )
                sp_cur = sp_next
                for s in range(NSUB):
                    ps = slice(s * 512, (s + 1) * 512)
                    nc.tensor.matmul(
                        U1[:, ps], mt16[:, j, 0:128], e16[:, ps],
                        start=(j == 0), stop=(j == NJT - 1), skip_group_check=True,
                    )
                    nc.tensor.matmul(
                        U2[:, ps], mt16[:, j, 128:161], e16[:, ps],
                        start=(j == 0), stop=(j == NJT - 1), skip_group_check=True,
                    )
            # norm head: stage U in SBUF + reciprocal of the denominator.
            # No PE instructions here — the PE queue is FIFO and must flow
            # straight into the next half's attention matmuls.
            U1c = work.tile([128, HALF], F32, tag="U1c")
            nc.vector.tensor_copy(out=U1c, in_=U1)
            U2c = work.tile([33, HALF], F32, tag="U2c")
            nc.vector.tensor_copy(out=U2c, in_=U2)
            U1n.append(U1c)
            U2n.append(U2c)
            if h == 0:
                # h0's reciprocal runs on DVE while h1's attention occupies
                # the PE; h1's is emitted at the end of h0's tail so it does
                # not block h0's normalization muls in the DVE FIFO
                rr = work.tile([1, HALF], F32, tag="rr")
                rr16 = work.tile([1, HALF], F16, tag="rr16")
                for sx in range(NSUB):
                    ps = slice(sx * 512, (sx + 1) * 512)
                    nc.vector.reciprocal(out=rr[:, ps], in_=U2c[32:33, ps])
                    nc.vector.tensor_copy(out=rr16[:, ps], in_=rr[:, ps])
                rr16n.append(rr16)

        # ---- gating + store, after both attention halves ------------------
        # (the PE queue is FIFO: tails must come after all attention matmuls
        # so the normalization chains overlap attention instead of stalling).
        # Both the contraction (g) and output (k) dims use the four
        # partition-aligned chunks [0:128],[128:150],[150:278],[278:300] so
        # x.T and U feed the matmul and the gate multiply with no
        # partition-shifting DMAs at all.
        for h in range(2):
            hs = slice(h * HALF, (h + 1) * HALF)
            # norm tail: PE broadcast of 1/denom, then normalize U
            bc = psb.tile([128, HALF], F32, tag="big")
            for sx in range(NSUB):
                ps = slice(sx * 512, (sx + 1) * 512)
                nc.tensor.matmul(
                    bc[:, ps], ones16, rr16n[h][:, ps],
                    start=True, stop=True, skip_group_check=True,
                )
            RCraw = work.tile([128, HALF], F32, tag="RCraw")
            nc.vector.tensor_mul(out=RCraw, in0=U1n[h], in1=bc)
            RDraw = work.tile([D - 128, HALF], F32, tag="RDraw")
            nc.vector.tensor_mul(
                out=RDraw, in0=U2n[h][0 : D - 128, :], in1=bc[0 : D - 128, :]
            )
            RC16 = work.tile([128, HALF], F16, tag="RC16")
            nc.vector.tensor_copy(out=RC16, in_=RCraw)
            RD16 = work.tile([D - 128, HALF], F16, tag="RD16")
            nc.vector.tensor_copy(out=RD16, in_=RDraw)
            # f32 res.T by output chunk for the final gate multiply:
            # partition-shifting DMAs split across the two HWDGE queues
            R1 = work.tile([128, HALF], F32, tag="R1f")
            R2 = work.tile([G - 256, HALF], F32, tag="R2f")
            nc.vector.tensor_copy(out=R1[0 : D - 128, :], in_=xT2[:, hs])
            nc.sync.dma_start(out=R1[D - 128 : 128, :], in_=RCraw[0 : 256 - D, :])
            nc.scalar.dma_start(out=R2[0 : D - 128, :], in_=RCraw[256 - D : 128, :])
            nc.scalar.dma_start(out=R2[D - 128 : G - 256, :], in_=RDraw)
            if h == 0:
                rr = work.tile([1, HALF], F32, tag="rr")
                rr16 = work.tile([1, HALF], F16, tag="rr16")
                for sx in range(NSUB):
                    ps = slice(sx * 512, (sx + 1) * 512)
                    nc.vector.reciprocal(out=rr[:, ps], in_=U2n[1][32:33, ps])
                    nc.vector.tensor_copy(out=rr16[:, ps], in_=rr[:, ps])
                rr16n.append(rr16)
            res16 = [xT116[:, hs], xT216[:, hs], RC16, RD16]
            resf = [xT1[:, hs], R1, R2]
            oT = [
                work.tile([128, HALF], F32, tag="oT0", name="oT0"),
                work.tile([128, HALF], F32, tag="oT1", name="oT1"),
                work.tile([G - 256, HALF], F32, tag="oT2", name="oT2"),
            ]
            for kc, (k0, k1) in enumerate(kranges):
                kw = k1 - k0
                gp_kind = ["u1", "u2", "big"][kc]
                gp_pool = psb if gp_kind == "big" else psu
                gp = gp_pool.tile([kw, HALF], F32, tag=gp_kind, name="gp")
                for sx in range(NSUB):
                    ps = slice(sx * 512, (sx + 1) * 512)
                    for gc in range(4):
                        nc.tensor.matmul(
                            gp[:, ps], wg16[gc][:, k0:k1], res16[gc][:, ps],
                            start=(gc == 0), stop=(gc == 3),
                            skip_group_check=True,
                        )
                gs = work.tile([kw, HALF], F32, tag="gs", bufs=3)
                nc.scalar.activation(
                    out=gs, in_=gp, func=Sigmoid, bias=bg_sb[kc], scale=1.0
                )
                nc.vector.tensor_mul(out=oT[kc], in0=gs, in1=resf[kc])

            for c in range(NCH // 2):
                sl = slice(c * 128, (c + 1) * 128)
                op = psb.tile([128, 320], F32, tag="big", name="op")
                nc.tensor.transpose(op[:, 0:128], oT[0][:, sl], ident)
                nc.tensor.transpose(op[:, 128:256], oT[1][:, sl], ident)
                nc.tensor.transpose(
                    op[:, 256:G], oT[2][:, sl], ident[: G - 256, : G - 256]
                )
                onat = work.tile([128, G], F32, tag="onat", bufs=4)
                nc.vector.tensor_copy(out=onat, in_=op[:, 0:G])
                # alternate HWDGE queues so the final stores drain in parallel
                dq = nc.sync if c % 2 == 0 else nc.scalar
                dq.dma_start(out=o_re[h * (NCH // 2) + c], in_=onat)


_NC_CACHE = None


def _build_nc():
    global _NC_CACHE
    if _NC_CACHE is not None:
        return _NC_CACHE
    nc = bacc.Bacc(None, target_bir_lowering=False, debug=False)
    x_d = nc.dram_tensor("x", [JX, D], F32, kind="ExternalInput")
    m_d = nc.dram_tensor("m", [JM, D], F32, kind="ExternalInput")
    mask_d = nc.dram_tensor("mask", [JM], I32, kind="ExternalInput")
    wi_d = nc.dram_tensor("Wi", [D, H], F32, kind="ExternalInput")
    bi_d = nc.dram_tensor("bi", [H], F32, kind="ExternalInput")
    wm_d = nc.dram_tensor("Wm", [D, H], F32, kind="ExternalInput")
    bm_d = nc.dram_tensor("bm", [H], F32, kind="ExternalInput")
    wg_d = nc.dram_tensor("Wg", [G, G], F32, kind="ExternalInput")
    bg_d = nc.dram_tensor("bg", [G], F32, kind="ExternalInput")
    o_d = nc.dram_tensor("out", [JX, G], F32, kind="ExternalOutput")
    with tile.TileContext(nc) as tc:
        _body(tc, x_d, m_d, mask_d, wi_d, bi_d, wm_d, bm_d, wg_d, bg_d, o_d)
    nc.finalize()
    _NC_CACHE = nc
    return nc


def _in_maps(inputs, memory, mask, Wi, bi, Wm, bm, Wg, bg):
    maps = []
    for b in range(B):
        maps.append(
            {
                "x": np.ascontiguousarray(inputs[b], dtype=np.float32),
                "m": np.ascontiguousarray(memory[b], dtype=np.float32),
                "mask": np.ascontiguousarray(mask[b], dtype=np.int32),
                "Wi": np.ascontiguousarray(Wi, dtype=np.float32),
                "bi": np.ascontiguousarray(bi, dtype=np.float32),
                "Wm": np.ascontiguousarray(Wm, dtype=np.float32),
                "bm": np.ascontiguousarray(bm, dtype=np.float32),
                "Wg": np.ascontiguousarray(Wg, dtype=np.float32),
                "bg": np.ascontiguousarray(bg, dtype=np.float32),
            }
        )
    return maps


def run_spmd(inputs, memory, mask, Wi, bi, Wm, bm, Wg, bg, **spmd_kwargs):
    """Run the kernel across 8 cores; returns the BassKernelResults."""
    nc = _build_nc()
    maps = _in_maps(
        np.asarray(inputs), np.asarray(memory), np.asarray(mask),
        np.asarray(Wi), np.asarray(bi), np.asarray(Wm), np.asarray(bm),
        np.asarray(Wg), np.asarray(bg),
    )
    return run_bass_kernel_spmd(nc, maps, list(range(B)), **spmd_kwargs)


def kernel(inputs, memory, mask, Wi, bi, Wm, bm, Wg, bg):
    res = run_spmd(inputs, memory, mask, Wi, bi, Wm, bm, Wg, bg)
    out = np.stack([res.results[b]["out"] for b in range(B)], axis=0)
    return out.astype(np.float32)



# revision 11
# speedup vs baseline: 1.0280x; 1.0280x over previous
"""Trainium2 Bass kernel for nn_DotAttention (B=8 data-parallel over 8 cores).

fp8(e4m3) + DoubleRow redesign. Per core (one batch element):
  xp.T = relu(Wi.T @ x.T + bi)     [96, 2048]   DR fp8, d packed 64x2(+pad)
  mp.T = relu(Wm.T @ m.T + bm)     [96, 2048]   DR fp8
  S.T[jm,jx] = mp.T(:,jtile) . xp.T             DR fp8, h packed 64+32zero
  e8 = exp(S.T*scale + maskbias)                ACT -> fp8 directly
  U[jx, 0:151] = sum_jm e8.T @ [m|1]            DR fp8, NATURAL layout:
                                                 denom lands in col 150
  U16n = U * rcp(denom)                         per-partition scalar (DVE)
  gate.T = sigmoid(Wg.T @ res.T + bg)           DR fp8 (res.T slots shared
                                                 with proj moving operand)
  out = transpose(gate.T) * [x | U16n]          natural, fp16 transposes
All DR slot strides are %16; zero-padded slots make uneven dims (150, 96,
300) fit the 2-ktile DoubleRow interleave. Weights are pre-scaled (Wi/Wm x8,
Wg x16) to keep fp8 quantization in the normal range; the inverse scale is
folded into the ACT activations.
"""

import contextlib
import math

import numpy as np

import concourse.bass as bass
import concourse.mybir as mybir
import concourse.tile as tile
from concourse import bacc
from concourse.bass_utils import run_bass_kernel_spmd
from concourse.masks import make_identity

F32 = mybir.dt.float32
F16 = mybir.dt.float16
F8 = mybir.dt.float8e4
I32 = mybir.dt.int32
DR = mybir.MatmulPerfMode.DoubleRow

B = 8
JX = 2048
JM = 2048
D = 150
H = 96
G = 300
NJT = 16          # jm tiles of 128
NCH = 16          # jx chunks of 128
HALF = 1024
NSUB = HALF // 512
SCALE = 1.0 / math.sqrt(float(H))
NEG_BIG = 1.0e30
WSCALE = 8.0      # Wi/Wm pre-scale for fp8 range
GSCALE = 16.0     # Wg pre-scale


def _body(tc, x_d, m_d, mask_d, wi_d, bi_d, wm_d, bm_d, wg_d, bg_d, o_d):
    nc = tc.nc
    Relu = mybir.ActivationFunctionType.Relu
    Exp = mybir.ActivationFunctionType.Exp
    Sigmoid = mybir.ActivationFunctionType.Sigmoid
    MUL = mybir.AluOpType.mult
    SUB = mybir.AluOpType.subtract

    with contextlib.ExitStack() as ctx:
        const = ctx.enter_context(tc.tile_pool(name="const", bufs=1))
        work = ctx.enter_context(tc.tile_pool(name="work", bufs=2))
        epool = ctx.enter_context(tc.tile_pool(name="epool", bufs=2))
        # PSUM: sp 2x[128,1024]f32 (4 banks) + U [128,8,152]f32 (3 banks)
        # + tail bank shared by preamble transposes / gp / tG
        psb = ctx.enter_context(tc.tile_pool(name="psb", bufs=2, space="PSUM"))
        pu = ctx.enter_context(tc.tile_pool(name="pu", bufs=1, space="PSUM"))
        pt = ctx.enter_context(tc.tile_pool(name="pt", bufs=1, space="PSUM"))

        ident8 = const.tile([128, 128], F8)
        make_identity(nc, ident8)
        ident16 = const.tile([128, 128], F16)
        make_identity(nc, ident16)
        ident32s = const.tile([NJT, NJT], F32)
        make_identity(nc, ident32s)

        # preload the exp table set (covers exp/relu/copy) during DMA wait
        dummy = const.tile([1, 1], F32)
        nc.scalar.activation(out=dummy, in_=ident16[0:1, 0:1], func=Exp, scale=1.0)

        # ---- weights (small, first on scalar queue) ----------------------
        wstage = const.tile([128, 2 * H], F32)
        nc.scalar.dma_start(out=wstage[:, 0:H], in_=wi_d[0:128, :])
        nc.scalar.dma_start(out=wstage[:, H : 2 * H], in_=wm_d[0:128, :])
        wstage2 = const.tile([D - 128, 2 * H], F32)
        nc.scalar.dma_start(out=wstage2[:, 0:H], in_=wi_d[128:D, :])
        nc.scalar.dma_start(out=wstage2[:, H : 2 * H], in_=wm_d[128:D, :])
        bi_sb = const.tile([H, 1], F32)
        nc.scalar.dma_start(out=bi_sb, in_=bi_d.rearrange("(n one) -> n one", one=1))
        bm_sb = const.tile([H, 1], F32)
        nc.scalar.dma_start(out=bm_sb, in_=bm_d.rearrange("(n one) -> n one", one=1))

        # Wi8/Wm8 [64, 4, 96]: d-slots 0..63 | 64..127 | 128..149+0 | 0
        wi8 = const.tile([64, 4, 96], F8)
        wm8 = const.tile([64, 4, 96], F8)
        for w8, col in ((wi8, 0), (wm8, H)):
            nc.vector.memset(w8[:, 2:4, :], 0.0)
            nc.vector.tensor_scalar(
                out=w8[:, 0, :], in0=wstage[0:64, col : col + H],
                scalar1=WSCALE, scalar2=None, op0=MUL)
            nc.vector.tensor_scalar(
                out=w8[:, 1, :], in0=wstage[64:128, col : col + H],
                scalar1=WSCALE, scalar2=None, op0=MUL)
            nc.vector.tensor_scalar(
                out=w8[0 : D - 128, 2, :], in0=wstage2[:, col : col + H],
                scalar1=WSCALE, scalar2=None, op0=MUL)

        # ---- mask -> additive exp bias [128, NJT] ------------------------
        mask_sb = const.tile([NJT, 128], I32)
        nc.sync.dma_start(out=mask_sb, in_=mask_d.rearrange("(n p) -> n p", p=128))
        maskf = const.tile([NJT, 128], F32)
        nc.vector.tensor_copy(out=maskf, in_=mask_sb)
        nc.vector.tensor_scalar(
            out=maskf, in0=maskf, scalar1=1.0, scalar2=NEG_BIG,
            op0=SUB, op1=MUL)
        mb_ps = pt.tile([128, NJT], F32, tag="tail", name="mbps")
        nc.tensor.transpose(mb_ps, maskf, ident32s)
        maskbias = const.tile([128, NJT], F32)
        nc.vector.tensor_copy(out=maskbias, in_=mb_ps)

        # ---- inputs: stream in 4 groups of 4 tiles each ------------------
        x_nat = const.tile([128, NCH, D], F32)
        m_nat = const.tile([128, NJT, D], F32)
        x_re = x_d.rearrange("(n p) d -> p n d", p=128)
        m_re = m_d.rearrange("(n p) d -> p n d", p=128)
        # fp8 naturals: mt8 cols 0..149=m, 150=1.0 (denominator), rest 0
        mt8 = const.tile([128, NJT, 176], F8)
        nc.gpsimd.memset(mt8[:, :, D:176], 0.0)
        nc.gpsimd.memset(mt8[:, :, 150:151], 1.0)
        x8 = const.tile([128, NCH, 152], F8)

        # shared T-layout fp8 slots [64, 8, 2048]:
        #  0: x.T d0..63    1: x.T d64..127   2: x.T d128..149 + 0
        #  3: U.T 0..63     4: U.T 64..127    5: U.T 128..149 + 0
        #  (3..5 are written per-half during attention; at proj time slot 3
        #   is all-zero and serves as the DR padding slot)
        xrT8 = const.tile([64, 8, JX], F8)
        nc.vector.memset(xrT8[:, 2, :], 0.0)
        nc.vector.memset(xrT8[:, 3, :], 0.0)
        nc.vector.memset(xrT8[:, 5, :], 0.0)
        mT8 = const.tile([64, 4, JM], F8)
        nc.vector.memset(mT8[:, 2, :], 0.0)
        nc.vector.memset(mT8[:, 3, :], 0.0)

        for g in range(4):
            gs4 = slice(g * 4, (g + 1) * 4)
            nc.sync.dma_start(out=m_nat[:, gs4, :], in_=m_re[:, gs4, :])
            nc.scalar.dma_start(out=x_nat[:, gs4, :], in_=x_re[:, gs4, :])
            # casts: m on gpsimd, x on vector
            nc.gpsimd.tensor_copy(out=mt8[:, gs4, 0:D], in_=m_nat[:, gs4, :])
            nc.gpsimd.tensor_copy(out=x8[:, gs4, 0:D], in_=x_nat[:, gs4, :])
            # fp8 transposes (stride-2 psum out), then packed copies to slots
            for src8, dstT, eng in ((mt8, mT8, nc.vector), (x8, xrT8, nc.scalar)):
                pA = pt.tile([128, 4, 256, 2], F8, tag="tail", name="pA")
                for i in range(4):
                    c = g * 4 + i
                    nc.tensor.transpose(
                        pA[0:64, i, 0:128, 0], src8[:, c, 0:64], ident8)
                    nc.tensor.transpose(
                        pA[64:128, i, 0:128, 0], src8[:, c, 64:128], ident8)
                    nc.tensor.transpose(
                        pA[0:22, i, 128:256, 0], src8[:, c, 128:D], ident8)
                gcols = slice(g * 512, (g + 1) * 512)
                cp = eng.tensor_copy if eng is nc.vector else eng.copy
                cp(out=dstT[:, 0, gcols], in_=pA[0:64, :, 0:128, 0])
                cp(out=dstT[:, 1, gcols], in_=pA[64:128, :, 0:128, 0])
                cp(out=dstT[0:22, 2, gcols], in_=pA[0:22, :, 128:256, 0])

        # Wg/bg late (needed only at gate time)
        # Wg8 [64, 6, 304]: g-slots x d0..63|d64..127|d128..149+0|
        #                            U0..63|U64..127|U128..149+0
        wg8 = const.tile([64, 6, 304], F8)
        nc.gpsimd.memset(wg8[:, 2, :], 0.0)
        nc.gpsimd.memset(wg8[:, 5, :], 0.0)
        for sl, (g0, g1) in enumerate([(0, 64), (64, 128), (128, 150),
                                       (150, 214), (214, 278), (278, 300)]):
            wst = const.tile([g1 - g0, G], F32, tag=f"wgst_{sl}", name=f"wgst{sl}")
            nc.sync.dma_start(out=wst, in_=wg_d[g0:g1, :])
            nc.gpsimd.tensor_scalar(
                out=wg8[0 : g1 - g0, sl, 0:G], in0=wst,
                scalar1=GSCALE, scalar2=None, op0=MUL)
        bg_sb = []
        for gi, (k0, k1) in enumerate([(0, 128), (128, 256), (256, G)]):
            t = const.tile([k1 - k0, 1], F32, tag=f"bg_{gi}", name=f"bg{gi}")
            nc.sync.dma_start(
                out=t, in_=bg_d[k0:k1].rearrange("(n one) -> n one", one=1))
            bg_sb.append(t)

        # ---- projections: xpT8/mpT8 [64, 2, 2048] (h 0..63 | 64..95+0) ---
        xpT8 = const.tile([64, 2, JX], F8)
        nc.vector.memset(xpT8[32:64, 1, :], 0.0)
        mpT8 = const.tile([64, 2, JM], F8)
        nc.vector.memset(mpT8[32:64, 1, :], 0.0)
        for w8, b_sb, srcT, dst in (
            (wm8, bm_sb, mT8, mpT8), (wi8, bi_sb, xrT8, xpT8)):
            for ch in range(2):
                cs = slice(ch * HALF, (ch + 1) * HALF)
                pp = psb.tile([128, HALF], F32, tag="big", name="pp")
                for sx in range(NSUB):
                    ss = slice(ch * HALF + sx * 512, ch * HALF + (sx + 1) * 512)
                    ps = slice(sx * 512, (sx + 1) * 512)
                    for pr in range(2):
                        nc.tensor.matmul(
                            pp[0:H, ps], w8[:, 2 * pr : 2 * pr + 2, :],
                            srcT[:, 2 * pr : 2 * pr + 2, ss],
                            start=(pr == 0), stop=(pr == 1),
                            perf_mode=DR, skip_group_check=True)
                nc.scalar.activation(
                    out=dst[:, 0, cs], in_=pp[0:64, :], func=Relu,
                    bias=b_sb[0:64, :], scale=1.0 / WSCALE)
                nc.scalar.activation(
                    out=dst[0:32, 1, cs], in_=pp[64:H, :], func=Relu,
                    bias=b_sb[64:H, :], scale=1.0 / WSCALE)

        # ---- attention per jx half ---------------------------------------
        U16n = const.tile([128, NCH, 152], F16)
        rcp_all = const.tile([128, NCH], F32)
        for h in range(2):
            hs = slice(h * HALF, (h + 1) * HALF)
            Up = pu.tile([128, 8, 152], F32, tag="U", name="Up")

            def emit_scores(j):
                sp = psb.tile([128, HALF], F32, tag="big", name="sp")
                for sx in range(NSUB):
                    ss = slice(h * HALF + sx * 512, h * HALF + (sx + 1) * 512)
                    nc.tensor.matmul(
                        sp[:, sx * 512 : (sx + 1) * 512],
                        mpT8[:, :, j * 128 : (j + 1) * 128],
                        xpT8[:, :, ss],
                        start=True, stop=True,
                        perf_mode=DR, skip_group_check=True)
                return sp

            sp_cur = emit_scores(0)
            e_cur = epool.tile([128, 2, HALF], F8, tag="e8", name="e8")
            for t in range(NJT // 2):
                for s in range(2):
                    j = 2 * t + s
                    sp_next = emit_scores(j + 1) if j + 1 < NJT else None
                    nc.scalar.activation(
                        out=e_cur[:, s, :], in_=sp_cur, func=Exp,
                        bias=maskbias[:, j : j + 1], scale=SCALE)
                    sp_cur = sp_next
                for c in range(8):
                    nc.tensor.matmul(
                        Up[:, c, 0:151],
                        e_cur[:, :, c * 128 : (c + 1) * 128],
                        mt8[:, 2 * t : 2 * t + 2, 0:151],
                        start=(t == 0), stop=(t == NJT // 2 - 1),
                        perf_mode=DR, skip_group_check=True)
                if t < NJT // 2 - 1:
                    e_cur = epool.tile([128, 2, HALF], F8, tag="e8", name="e8")

            # normalize (DVE only; overlaps the other half's attention)
            hc = slice(h * 8, h * 8 + 8)
            den = work.tile([128, 8], F32, tag="den")
            nc.vector.tensor_copy(out=den, in_=Up[:, :, 150])
            nc.vector.reciprocal_approx_fast(
                out=rcp_all[:, hc], in_=den)
            for c in range(8):
                nc.vector.tensor_scalar(
                    out=U16n[:, h * 8 + c, 0:D], in0=Up[:, c, 0:D],
                    scalar1=rcp_all[:, h * 8 + c : h * 8 + c + 1],
                    scalar2=None, op0=MUL)

        # ---- U.T transposes into xrT8 slots 3..5 (fp16 in, fp8 out) ------
        for g in range(4):
            pA = pt.tile([128, 4, 256], F16, tag="tail", name="pUA")
            for i in range(4):
                c = g * 4 + i
                nc.tensor.transpose(
                    pA[0:64, i, 0:128], U16n[:, c, 0:64], ident16)
                nc.tensor.transpose(
                    pA[64:128, i, 0:128], U16n[:, c, 64:128], ident16)
                nc.tensor.transpose(
                    pA[0:22, i, 128:256], U16n[:, c, 128:D], ident16)
            gcols = slice(g * 512, (g + 1) * 512)
            nc.vector.tensor_copy(out=xrT8[:, 3, gcols], in_=pA[0:64, :, 0:128])
            nc.vector.tensor_copy(out=xrT8[:, 4, gcols], in_=pA[64:128, :, 0:128])
            nc.vector.tensor_copy(out=xrT8[0:22, 5, gcols], in_=pA[0:22, :, 128:256])

        # ---- gate + output ----------------------------------------------
        o_re = o_d.rearrange("(n p) k -> p n k", p=128)
        kranges = [(0, 128), (128, 256), (256, G)]
        for h in range(2):
            gs16 = [
                work.tile([128, HALF], F16, tag="gsA", name="gsA"),
                work.tile([128, HALF], F16, tag="gsB", name="gsB"),
                work.tile([44, HALF], F16, tag="gsC", name="gsC"),
            ]
            for kc, (k0, k1) in enumerate(kranges):
                kw = k1 - k0
                for sx in range(NSUB):
                    ss = slice(h * HALF + sx * 512, h * HALF + (sx + 1) * 512)
                    ps = slice(sx * 512, (sx + 1) * 512)
                    gp = psb.tile([128, 512], F32, tag="big", name="gp")
                    for pr in range(3):
                        nc.tensor.matmul(
                            gp[0:kw, :], wg8[:, 2 * pr : 2 * pr + 2, k0:k1],
                            xrT8[:, 2 * pr : 2 * pr + 2, ss],
                            start=(pr == 0), stop=(pr == 2),
                            perf_mode=DR, skip_group_check=True)
                    nc.scalar.activation(
                        out=gs16[kc][:, ps], in_=gp[0:kw, :], func=Sigmoid,
                        bias=bg_sb[kc], scale=1.0 / GSCALE)
            # transpose gate to natural, multiply, store
            for cp in range(4):  # pairs of jx chunks
                tG = psb.tile([128, 2, 304], F16, tag="big", name="tG")
                for i in range(2):
                    lc = slice((cp * 2 + i) * 128, (cp * 2 + i + 1) * 128)
                    nc.tensor.transpose(
                        tG[:, i, 0:128], gs16[0][:, lc], ident16)
                    nc.tensor.transpose(
                        tG[:, i, 128:256], gs16[1][:, lc], ident16)
                    nc.tensor.transpose(
                        tG[:, i, 256:300], gs16[2][:, lc], ident16[0:44, 0:44])
                c2 = slice(h * 8 + cp * 2, h * 8 + cp * 2 + 2)
                onat = work.tile([128, 2, G], F32, tag="onat", bufs=4)
                eng = nc.vector
                eng.tensor_tensor(
                    out=onat[:, :, 0:D], in0=tG[:, :, 0:D],
                    in1=x_nat[:, c2, :], op=MUL)
                eng.tensor_tensor(
                    out=onat[:, :, D:G], in0=tG[:, :, D:G],
                    in1=U16n[:, c2, 0:D], op=MUL)
                dq = nc.sync if cp % 2 == 0 else nc.gpsimd
                dq.dma_start(out=o_re[:, c2, :], in_=onat)


_NC_CACHE = None


def _build_nc():
    global _NC_CACHE
    if _NC_CACHE is not None:
        return _NC_CACHE
    nc = bacc.Bacc(None, target_bir_lowering=False, debug=False)
    x_d = nc.dram_tensor("x", [JX, D], F32, kind="ExternalInput")
    m_d = nc.dram_tensor("m", [JM, D], F32, kind="ExternalInput")
    mask_d = nc.dram_tensor("mask", [JM], I32, kind="ExternalInput")
    wi_d = nc.dram_tensor("Wi", [D, H], F32, kind="ExternalInput")
    bi_d = nc.dram_tensor("bi", [H], F32, kind="ExternalInput")
    wm_d = nc.dram_tensor("Wm", [D, H], F32, kind="ExternalInput")
    bm_d = nc.dram_tensor("bm", [H], F32, kind="ExternalInput")
    wg_d = nc.dram_tensor("Wg", [G, G], F32, kind="ExternalInput")
    bg_d = nc.dram_tensor("bg", [G], F32, kind="ExternalInput")
    o_d = nc.dram_tensor("out", [JX, G], F32, kind="ExternalOutput")
    with tile.TileContext(nc) as tc:
        _body(tc, x_d, m_d, mask_d, wi_d, bi_d, wm_d, bm_d, wg_d, bg_d, o_d)
    nc.finalize()
    _NC_CACHE = nc
    return nc


def _in_maps(inputs, memory, mask, Wi, bi, Wm, bm, Wg, bg):
    maps = []
    for b in range(B):
        maps.append(
            {
                "x": np.ascontiguousarray(inputs[b], dtype=np.float32),
                "m": np.ascontiguousarray(memory[b], dtype=np.float32),
                "mask": np.ascontiguousarray(mask[b], dtype=np.int32),
                "Wi": np.ascontiguousarray(Wi, dtype=np.float32),
                "bi": np.ascontiguousarray(bi, dtype=np.float32),
                "Wm": np.ascontiguousarray(Wm, dtype=np.float32),
                "bm": np.ascontiguousarray(bm, dtype=np.float32),
                "Wg": np.ascontiguousarray(Wg, dtype=np.float32),
                "bg": np.ascontiguousarray(bg, dtype=np.float32),
            }
        )
    return maps


def run_spmd(inputs, memory, mask, Wi, bi, Wm, bm, Wg, bg, **spmd_kwargs):
    """Run the kernel across 8 cores; returns the BassKernelResults."""
    nc = _build_nc()
    maps = _in_maps(
        np.asarray(inputs), np.asarray(memory), np.asarray(mask),
        np.asarray(Wi), np.asarray(bi), np.asarray(Wm), np.asarray(bm),
        np.asarray(Wg), np.asarray(bg),
    )
    return run_bass_kernel_spmd(nc, maps, list(range(B)), **spmd_kwargs)


def kernel(inputs, memory, mask, Wi, bi, Wm, bm, Wg, bg):
    res = run_spmd(inputs, memory, mask, Wi, bi, Wm, bm, Wg, bg)
    out = np.stack([res.results[b]["out"] for b in range(B)], axis=0)
    return out.astype(np.float32)


# revision 16
# speedup vs baseline: 1.3970x; 1.3590x over previous
"""Trainium2 Bass kernel for nn_DotAttention (B=8 data-parallel over 8 cores).

fp8(e4m3) + DoubleRow redesign. Per core (one batch element):
  xp.T = relu(Wi.T @ x.T + bi)     [96, 2048]   DR fp8, d packed 64x2(+pad)
  mp.T = relu(Wm.T @ m.T + bm)     [96, 2048]   DR fp8
  S.T[jm,jx] = mp.T(:,jtile) . xp.T             DR fp8, h packed 64+32zero
  e8 = exp(S.T*scale + maskbias)                ACT -> fp8 directly
  U[jx, 0:151] = sum_jm e8.T @ [m|1]            DR fp8, NATURAL layout:
                                                 denom lands in col 150
  U16n = U * rcp(denom)                         per-partition scalar (DVE)
  gate.T = sigmoid(Wg.T @ res.T + bg)           DR fp8 (res.T slots shared
                                                 with proj moving operand)
  out = transpose(gate.T) * [x | U16n]          natural, fp16 transposes
All DR slot strides are %16; zero-padded slots make uneven dims (150, 96,
300) fit the 2-ktile DoubleRow interleave. Weights are pre-scaled (Wi/Wm x8,
Wg x16) to keep fp8 quantization in the normal range; the inverse scale is
folded into the ACT activations.
"""

import contextlib
import math

import numpy as np

import concourse.bass as bass
import concourse.mybir as mybir
import concourse.tile as tile
from concourse import bacc
from concourse.bass_utils import run_bass_kernel_spmd
from concourse.masks import make_identity

F32 = mybir.dt.float32
F16 = mybir.dt.float16
F8 = mybir.dt.float8e4
I32 = mybir.dt.int32
DR = mybir.MatmulPerfMode.DoubleRow

B = 8
JX = 2048
JM = 2048
D = 150
H = 96
G = 300
NJT = 16          # jm tiles of 128
NCH = 16          # jx chunks of 128
HALF = 1024
NSUB = HALF // 512
SCALE = 1.0 / math.sqrt(float(H))
NEG_BIG = 1.0e30
WSCALE = 8.0      # Wi/Wm pre-scale for fp8 range
GSCALE = 16.0     # Wg pre-scale


def _body(tc, x_d, m_d, mask_d, wi_d, bi_d, wm_d, bm_d, wg_d, bg_d, o_d):
    nc = tc.nc
    Relu = mybir.ActivationFunctionType.Relu
    Exp = mybir.ActivationFunctionType.Exp
    Sigmoid = mybir.ActivationFunctionType.Sigmoid
    MUL = mybir.AluOpType.mult
    SUB = mybir.AluOpType.subtract

    with contextlib.ExitStack() as ctx:
        const = ctx.enter_context(tc.tile_pool(name="const", bufs=1))
        work = ctx.enter_context(tc.tile_pool(name="work", bufs=2))
        epool = ctx.enter_context(tc.tile_pool(name="epool", bufs=2))
        psb = ctx.enter_context(tc.tile_pool(name="psb", bufs=2, space="PSUM"))
        pu = ctx.enter_context(tc.tile_pool(name="pu", bufs=1, space="PSUM"))
        pt = ctx.enter_context(tc.tile_pool(name="pt", bufs=1, space="PSUM"))

        ident16 = const.tile([128, 128], F16)
        make_identity(nc, ident16)
        ident32s = const.tile([NJT, NJT], F32)
        make_identity(nc, ident32s)

        # preload the exp table set (covers exp/relu/copy) during DMA wait
        dummy = const.tile([1, 1], F32)
        nc.scalar.activation(out=dummy, in_=ident16[0:1, 0:1], func=Exp, scale=1.0)

        # ---- weights (small, first on scalar queue) ----------------------
        wstage = const.tile([128, 2 * H], F32)
        nc.scalar.dma_start(out=wstage[:, 0:H], in_=wi_d[0:128, :])
        nc.scalar.dma_start(out=wstage[:, H : 2 * H], in_=wm_d[0:128, :])
        wstage2 = const.tile([D - 128, 2 * H], F32)
        nc.scalar.dma_start(out=wstage2[:, 0:H], in_=wi_d[128:D, :])
        nc.scalar.dma_start(out=wstage2[:, H : 2 * H], in_=wm_d[128:D, :])
        bi_sb = const.tile([H, 1], F32)
        nc.scalar.dma_start(out=bi_sb, in_=bi_d.rearrange("(n one) -> n one", one=1))
        bm_sb = const.tile([H, 1], F32)
        nc.scalar.dma_start(out=bm_sb, in_=bm_d.rearrange("(n one) -> n one", one=1))
        wi16a = const.tile([128, H], F16)
        nc.vector.tensor_copy(out=wi16a, in_=wstage[:, 0:H])
        wi16b = const.tile([D - 128, H], F16)
        nc.vector.tensor_copy(out=wi16b, in_=wstage2[:, 0:H])
        wm16a = const.tile([128, H], F16)
        nc.vector.tensor_copy(out=wm16a, in_=wstage[:, H : 2 * H])
        wm16b = const.tile([D - 128, H], F16)
        nc.vector.tensor_copy(out=wm16b, in_=wstage2[:, H : 2 * H])

        # ---- mask -> additive exp bias [128, NJT] ------------------------
        mask_sb = const.tile([NJT, 128], I32)
        nc.sync.dma_start(out=mask_sb, in_=mask_d.rearrange("(n p) -> n p", p=128))
        maskf = const.tile([NJT, 128], F32)
        nc.vector.tensor_copy(out=maskf, in_=mask_sb)
        nc.vector.tensor_scalar(
            out=maskf, in0=maskf, scalar1=1.0, scalar2=NEG_BIG,
            op0=SUB, op1=MUL)
        mb_ps = pt.tile([128, NJT], F32, tag="tail", name="mbps")
        nc.tensor.transpose(mb_ps, maskf, ident32s)
        maskbias = const.tile([128, NJT], F32)
        nc.vector.tensor_copy(out=maskbias, in_=mb_ps)

        # ---- inputs: stream in, cast, transpose --------------------------
        x_nat = const.tile([128, NCH, D], F32)
        m_nat = const.tile([128, NJT, D], F32)
        x_re = x_d.rearrange("(n p) d -> p n d", p=128)
        m_re = m_d.rearrange("(n p) d -> p n d", p=128)
        x16 = const.tile([128, NCH, D], F16)
        m16 = const.tile([128, NJT, D], F16)
        # fp8 natural m for the U matmuls (cols 0..149 = m, 150 = 1.0)
        mt8 = const.tile([128, NJT, 176], F8)
        nc.gpsimd.memset(mt8[:, :, D:176], 0.0)
        nc.gpsimd.memset(mt8[:, :, 150:151], 1.0)

        pA2 = pt.tile([128, 4, 256], F16, tag="tail", name="pA2")
        xT16a = const.tile([128, JX], F16)
        xT16b = const.tile([D - 128, JX], F16)
        mT16a = const.tile([128, JM], F16)
        mT16b = const.tile([D - 128, JM], F16)

        for g in range(4):
            gs4 = slice(g * 4, (g + 1) * 4)
            nc.sync.dma_start(out=m_nat[:, gs4, :], in_=m_re[:, gs4, :])
            nc.scalar.dma_start(out=x_nat[:, gs4, :], in_=x_re[:, gs4, :])
            nc.scalar.copy(out=m16[:, gs4, :], in_=m_nat[:, gs4, :])
            nc.vector.tensor_copy(out=x16[:, gs4, :], in_=x_nat[:, gs4, :])
            nc.gpsimd.tensor_copy(out=mt8[:, gs4, 0:D], in_=m_nat[:, gs4, :])
            for si, (src16, dstA, dstB, eng) in enumerate((
                (m16, mT16a, mT16b, nc.scalar), (x16, xT16a, xT16b, nc.vector))):
                for half2 in range(2):
                    pp2 = (g * 4 + si * 2 + half2) % 2
                    pA = pA2[:, pp2 * 2 : pp2 * 2 + 2, :]
                    for i in range(2):
                        c = g * 4 + half2 * 2 + i
                        nc.tensor.transpose(
                            pA[:, i, 0:128], src16[:, c, 0:128], ident16)
                        nc.tensor.transpose(
                            pA[0 : D - 128, i, 128:256], src16[:, c, 128:D],
                            ident16)
                    gcols = slice(g * 512 + half2 * 256,
                                  g * 512 + (half2 + 1) * 256)
                    cpf = eng.tensor_copy if eng is nc.vector else eng.copy
                    cpf(out=dstA[:, gcols], in_=pA[:, :, 0:128])
                    cpf(out=dstB[:, gcols], in_=pA[0 : D - 128, :, 128:256])

        # Wg/bg late (needed only at gate time): moving operands, fp16.
        # 4 g-chunks; the last one carries an extra ones-row (g row 22)
        # paired with bg as the matching Wg row -> bias via matmul.
        wg16 = []
        for sl, (g0, g1) in enumerate([(0, 128), (128, 150), (150, 278),
                                       (278, 300)]):
            wst = const.tile([g1 - g0, G], F32, tag=f"wgst_{sl}", name=f"wgst{sl}")
            nc.sync.dma_start(out=wst, in_=wg_d[g0:g1, :])
            rows = (g1 - g0) if sl != 3 else 33
            w = const.tile([rows, G], F16, tag=f"wg16_{sl}", name=f"wg16{sl}")
            if sl == 3:
                nc.gpsimd.memset(w, 0.0)
            nc.gpsimd.tensor_copy(out=w[0 : g1 - g0, :], in_=wst)
            wg16.append(w)
        bgst = const.tile([1, G], F32, tag="bgst")
        nc.sync.dma_start(out=bgst, in_=bg_d.rearrange("(one n) -> one n", one=1))
        nc.gpsimd.tensor_copy(out=wg16[3][32:33, :], in_=bgst)

        # ---- projections -> xpT16/mpT16 [96, 2048] -----------------------
        xpT16 = const.tile([H, JX], F16)
        mpT16 = const.tile([H, JM], F16)
        for wa, wb, b_sb, srcA, srcB, dst in (
            (wm16a, wm16b, bm_sb, mT16a, mT16b, mpT16),
            (wi16a, wi16b, bi_sb, xT16a, xT16b, xpT16)):
            for ch in range(2):
                cs = slice(ch * HALF, (ch + 1) * HALF)
                pp = psb.tile([128, HALF], F32, tag="big", name="pp")
                for sx in range(NSUB):
                    ss = slice(ch * HALF + sx * 512, ch * HALF + (sx + 1) * 512)
                    ps = slice(sx * 512, (sx + 1) * 512)
                    nc.tensor.matmul(
                        pp[0:H, ps], wa, srcA[:, ss],
                        start=True, stop=False, skip_group_check=True)
                    nc.tensor.matmul(
                        pp[0:H, ps], wb, srcB[:, ss],
                        start=False, stop=True, skip_group_check=True)
                nc.scalar.activation(
                    out=dst[:, cs], in_=pp[0:H, :], func=Relu,
                    bias=b_sb, scale=1.0)

        # ---- attention per jx half ---------------------------------------
        U16n = const.tile([128, NCH, 152], F16)
        rcp_all = const.tile([128, NCH], F32)
        for h in range(2):
            hs = slice(h * HALF, (h + 1) * HALF)
            Up = pu.tile([128, 8, 152], F32, tag="U", name="Up")

            def emit_scores(j):
                sp = psb.tile([128, HALF], F32, tag="big", name="sp")
                for sx in range(NSUB):
                    ss = slice(h * HALF + sx * 512, h * HALF + (sx + 1) * 512)
                    nc.tensor.matmul(
                        sp[:, sx * 512 : (sx + 1) * 512],
                        mpT16[:, j * 128 : (j + 1) * 128], xpT16[:, ss],
                        start=True, stop=True, skip_group_check=True)
                return sp

            sp_cur = emit_scores(0)
            e_cur = epool.tile([128, 2, HALF], F8, tag="e8", name="e8")
            for t in range(NJT // 2):
                for s in range(2):
                    j = 2 * t + s
                    sp_next = emit_scores(j + 1) if j + 1 < NJT else None
                    nc.scalar.activation(
                        out=e_cur[:, s, :], in_=sp_cur, func=Exp,
                        bias=maskbias[:, j : j + 1], scale=SCALE)
                    sp_cur = sp_next
                for c in range(8):
                    nc.tensor.matmul(
                        Up[:, c, 0:151],
                        e_cur[:, :, c * 128 : (c + 1) * 128],
                        mt8[:, 2 * t : 2 * t + 2, 0:151],
                        start=(t == 0), stop=(t == NJT // 2 - 1),
                        perf_mode=DR, skip_group_check=True)
                if t < NJT // 2 - 1:
                    e_cur = epool.tile([128, 2, HALF], F8, tag="e8", name="e8")

            # normalize (DVE only; overlaps the other half's attention)
            hc = slice(h * 8, h * 8 + 8)
            den = work.tile([128, 8], F32, tag="den")
            nc.vector.tensor_copy(out=den, in_=Up[:, :, 150])
            nc.vector.reciprocal_approx_fast(out=rcp_all[:, hc], in_=den)
            for c in range(8):
                nc.vector.tensor_scalar(
                    out=U16n[:, h * 8 + c, 0:D], in0=Up[:, c, 0:D],
                    scalar1=rcp_all[:, h * 8 + c : h * 8 + c + 1],
                    scalar2=None, op0=MUL)

        # ---- U.T transposes -> uT16a/b (fp16) ----------------------------
        uT16a = const.tile([128, JX], F16)
        uT16b = const.tile([33, JX], F16)  # rows 0..21 = U.T tail, row 32 = ones
        nc.vector.memset(uT16b, 0.0)
        nc.vector.memset(uT16b[32:33, :], 1.0)
        for g in range(8):
            pA = pA2[:, (g % 2) * 2 : (g % 2) * 2 + 2, :]
            for i in range(2):
                c = g * 2 + i
                nc.tensor.transpose(
                    pA[:, i, 0:128], U16n[:, c, 0:128], ident16)
                nc.tensor.transpose(
                    pA[0 : D - 128, i, 128:256], U16n[:, c, 128:D], ident16)
            gcols = slice(g * 256, (g + 1) * 256)
            eng = nc.vector if g % 2 == 0 else nc.scalar
            cpf = eng.tensor_copy if eng is nc.vector else eng.copy
            cpf(out=uT16a[:, gcols], in_=pA[:, :, 0:128])
            cpf(out=uT16b[0 : D - 128, gcols], in_=pA[0 : D - 128, :, 128:256])

        # ---- gate (natural layout) + output ------------------------------
        o_re = o_d.rearrange("(n p) k -> p n k", p=128)
        gate16 = const.tile([128, NCH, G], F16)
        for c in range(NCH):
            cs = slice(c * 128, (c + 1) * 128)
            gp = psb.tile([128, 304], F32, tag="big", name="gp")
            for gi, (lhs, pstart) in enumerate((
                (xT16a[:, cs], True), (xT16b[:, cs], False),
                (uT16a[:, cs], False), (uT16b[:, cs], False))):
                nc.tensor.matmul(
                    gp[:, 0:G], lhs, wg16[gi],
                    start=(gi == 0), stop=(gi == 3), skip_group_check=True)
            nc.scalar.activation(
                out=gate16[:, c, :], in_=gp[:, 0:G], func=Sigmoid, scale=1.0)
        for cp in range(8):
            c2 = slice(cp * 2, cp * 2 + 2)
            onat = work.tile([128, 2, G], F32, tag="onat", bufs=4)
            eng = nc.vector if cp % 2 == 0 else nc.gpsimd
            eng.tensor_tensor(
                out=onat[:, :, 0:D], in0=gate16[:, c2, 0:D],
                in1=x_nat[:, c2, :], op=MUL)
            eng.tensor_tensor(
                out=onat[:, :, D:G], in0=gate16[:, c2, D:G],
                in1=U16n[:, c2, 0:D], op=MUL)
            dq = nc.sync if cp % 2 == 0 else nc.scalar
            dq.dma_start(out=o_re[:, c2, :], in_=onat)


_NC_CACHE = None


def _build_nc():
    global _NC_CACHE
    if _NC_CACHE is not None:
        return _NC_CACHE
    nc = bacc.Bacc(None, target_bir_lowering=False, debug=False)
    x_d = nc.dram_tensor("x", [JX, D], F32, kind="ExternalInput")
    m_d = nc.dram_tensor("m", [JM, D], F32, kind="ExternalInput")
    mask_d = nc.dram_tensor("mask", [JM], I32, kind="ExternalInput")
    wi_d = nc.dram_tensor("Wi", [D, H], F32, kind="ExternalInput")
    bi_d = nc.dram_tensor("bi", [H], F32, kind="ExternalInput")
    wm_d = nc.dram_tensor("Wm", [D, H], F32, kind="ExternalInput")
    bm_d = nc.dram_tensor("bm", [H], F32, kind="ExternalInput")
    wg_d = nc.dram_tensor("Wg", [G, G], F32, kind="ExternalInput")
    bg_d = nc.dram_tensor("bg", [G], F32, kind="ExternalInput")
    o_d = nc.dram_tensor("out", [JX, G], F32, kind="ExternalOutput")
    with tile.TileContext(nc) as tc:
        _body(tc, x_d, m_d, mask_d, wi_d, bi_d, wm_d, bm_d, wg_d, bg_d, o_d)
    nc.finalize()
    _NC_CACHE = nc
    return nc


def _in_maps(inputs, memory, mask, Wi, bi, Wm, bm, Wg, bg):
    maps = []
    for b in range(B):
        maps.append(
            {
                "x": np.ascontiguousarray(inputs[b], dtype=np.float32),
                "m": np.ascontiguousarray(memory[b], dtype=np.float32),
                "mask": np.ascontiguousarray(mask[b], dtype=np.int32),
                "Wi": np.ascontiguousarray(Wi, dtype=np.float32),
                "bi": np.ascontiguousarray(bi, dtype=np.float32),
                "Wm": np.ascontiguousarray(Wm, dtype=np.float32),
                "bm": np.ascontiguousarray(bm, dtype=np.float32),
                "Wg": np.ascontiguousarray(Wg, dtype=np.float32),
                "bg": np.ascontiguousarray(bg, dtype=np.float32),
            }
        )
    return maps


def run_spmd(inputs, memory, mask, Wi, bi, Wm, bm, Wg, bg, **spmd_kwargs):
    """Run the kernel across 8 cores; returns the BassKernelResults."""
    nc = _build_nc()
    maps = _in_maps(
        np.asarray(inputs), np.asarray(memory), np.asarray(mask),
        np.asarray(Wi), np.asarray(bi), np.asarray(Wm), np.asarray(bm),
        np.asarray(Wg), np.asarray(bg),
    )
    return run_bass_kernel_spmd(nc, maps, list(range(B)), **spmd_kwargs)


def kernel(inputs, memory, mask, Wi, bi, Wm, bm, Wg, bg):
    res = run_spmd(inputs, memory, mask, Wi, bi, Wm, bm, Wg, bg)
    out = np.stack([res.results[b]["out"] for b in range(B)], axis=0)
    return out.astype(np.float32)
